# revision 33
# baseline (speedup 1.0000x reference)
"""FFT-based DCT-II on 8 trn2 NeuronCores (v5.1: radix 32x128, dual-half).

Per core (256 rows, no h-split): Makhoul DCT->real-FFT with n = 32*n1' +
n2'. Stage 1 contracts n1' with K=128 (full array), 128 slots s = 2a+m
(65 cos + 63 sin, cos_64 parked in the sin_0 slot). Mid transpose
roundtrips DRAM: write legs fully contiguous (t_dram == t_sb layout),
read legs 512B runs into a DUAL-HALF t2 (group-halves on partition
halves -> all 16 SDMA engines + stage-2 row/col quadrant packing, 4
concurrent matmuls). fp16 output, 1KB-contiguous stores.

Layouts:
  x1[n1', 256 n2' + r] = v[r, 32 n1' + n2']
  t_sb = t_dram [128 s=(2a+m), 32 n2', 256 r]
  t2   [128 = (hf, m, n2'), 32 gg, 256 r]   (group a = 32 hf + gg)
  hh   [128 = (hf, m, n2'), 32 gg, 64 = (d, k2')]
  y    [16 qq, 2 hf, 2 d, 32 k2', 2 i, 256 r], a = 32 hf + 2 qq + i
"""

import numpy as np

N = 4096
R = 2048
RPC = 256

_state = {}


def _tables():
    n1 = np.arange(128, dtype=np.float64)
    f1 = np.zeros((128, 128))
    a_ = np.arange(64, dtype=np.float64)
    f1[:, 0::2] = np.cos(2 * np.pi * n1[:, None] * a_[None, :] / 128)
    f1[:, 3::2] = -np.sin(2 * np.pi * n1[:, None] * a_[None, 1:] / 128)
    f1[:, 1] = np.cos(np.pi * n1)  # cos_64 in the sin_0 slot
    f1_np = f1.astype(np.float16)  # [128, 128]

    n2 = np.arange(32, dtype=np.float64)[:, None]
    k2 = np.arange(32, dtype=np.float64)[None, :]
    hh = np.zeros((128, 32, 64))
    for hf in range(2):
        for gg in range(32):
            a = 32 * hf + gg
            for d in range(2):
                k1 = (a if d == 0 else 128 - a) if a >= 1 else (
                    0 if d == 0 else 64
                )
                kk = 128 * k2 + k1
                th = np.pi * kk * (4 * n2 + 1) / 8192
                cols = (32 * d + np.arange(32))[None, :]
                rows = np.arange(32)[:, None]
                base = 64 * hf
                if a == 0:
                    hh[base + 32 * d + rows, gg, cols] = np.cos(th)
                else:
                    sgn = 1.0 if d == 0 else -1.0
                    hh[base + rows, gg, cols] = np.cos(th)
                    hh[base + 32 + rows, gg, cols] = sgn * np.sin(th)
    hh_np = hh.astype(np.float16).copy()  # [128, 32, 64]

    k1_map = np.empty(128, dtype=np.int64)
    for a in range(64):
        for d in range(2):
            k1_map[2 * a + d] = (a if d == 0 else 128 - a) if a >= 1 else (
                0 if d == 0 else 64
            )
    return f1_np, hh_np, k1_map


def _build():
    import concourse.tile as tile
    from concourse import bacc, mybir

    f16 = mybir.dt.float16
    f32 = mybir.dt.float32

    nc = bacc.Bacc("TRN2", target_bir_lowering=False, debug=False, num_devices=8)
    x1_d = nc.dram_tensor("x1", [128, 8192], f16, kind="ExternalInput").ap()
    f1_d = nc.dram_tensor("f1", [128, 128], f16, kind="ExternalInput").ap()
    hh_d = nc.dram_tensor("hh", [128, 32, 64], f16, kind="ExternalInput").ap()
    y_d = nc.dram_tensor(
        "y", [16, 2, 2, 32, 2, 256], f16, kind="ExternalOutput"
    ).ap()

    with tile.TileContext(nc) as tc:
        with (
            tc.tile_pool(name="const", bufs=1) as const,
            tc.tile_pool(name="data", bufs=1) as data,
            tc.tile_pool(name="dram", bufs=1, space="DRAM") as dram,
            tc.tile_pool(name="ps1", bufs=4, space="PSUM") as ps1,
            tc.tile_pool(name="ps2", bufs=4, space="PSUM") as ps2,
            tc.tile_pool(name="ysb", bufs=12) as ysbp,
        ):
            f1_sb = const.tile([128, 128], f16)
            hh_sb = const.tile([128, 32, 64], f16)
            nc.sync.dma_start(f1_sb[:], f1_d)
            x1_g = []
            for g in range(8):
                xg = data.tile([128, 1024], f16, name=f"x1_{g}")
                nc.sync.dma_start(xg[:], x1_d[:, 1024 * g : 1024 * g + 1024])
                x1_g.append(xg)


            t_sb = data.tile([128, 32, 256], f16)
            t_dram = dram.tile([128, 32, 256], f16)  # same layout as t_sb
            t2 = data.tile([128, 32, 256], f16)  # (hf, m, n2'), gg, r

            # stage 1: per 2-n2' chunk one 1-bank psum tile, one matmul
            # [K=128, M=128, N=512], one copy out.
            cb = 0
            for g in range(8):
                for u in range(2):
                    ps = ps1.tile([128, 512], f32)
                    nc.tensor.matmul(
                        ps[:],
                        f1_sb[:],
                        x1_g[g][:, 512 * u : 512 * u + 512],
                        start=True,
                        stop=True,
                    )
                    n0 = 4 * g + 2 * u
                    dst = t_sb[:, n0 : n0 + 2, :]
                    src = ps[:].rearrange("p (n r) -> p n r", n=2)
                    if cb % 2 == 0:
                        nc.vector.tensor_copy(dst, src)
                    else:
                        nc.scalar.copy(dst, src)
                    cb += 1
                # transpose write legs: fully contiguous, eight 4-n2
                # waves, all on sync (SP is idle after the x issues;
                # keeps ACT free for copies)
                n0 = 4 * g
                nc.sync.dma_start(
                    t_dram[:, n0 : n0 + 4, :], t_sb[:, n0 : n0 + 4, :]
                )
                if g == 1:
                    # hh load on scalar, sequenced after two copies so
                    # its 0.5MB transfer misses both the x loads and the
                    # sync queue's write waves; lands well before stage 2
                    nc.scalar.dma_start(hh_sb[:], hh_d)

            # read legs: t2[64 hf + 32 m + n2', gg, r] =
            #            t_dram[2 (32 hf + gg) + m, n2', r]; 512B runs,
            # 32-partition dst, halves split across engine parity.
            t_dv = t_dram[:].rearrange("(a m) n r -> a m n r", m=2)
            for j4 in range(4):
                for hf in range(2):
                    for m in range(2):
                        src = t_dv[
                            32 * hf + 8 * j4 : 32 * hf + 8 * j4 + 8, m, :, :
                        ].rearrange("a n r -> n a r")
                        dst = t2[
                            64 * hf + 32 * m : 64 * hf + 32 * m + 32,
                            8 * j4 : 8 * j4 + 8,
                            :,
                        ]
                        if m == 0:
                            nc.sync.dma_start(dst, src)
                        else:
                            nc.scalar.dma_start(dst, src)

            # stage 2: per qq one 1-bank psum, 4 quadrant-packed matmuls
            # (group halves on row/col groups 0 and 64); copy; store.
            for qq in range(16):
                ps = ps2.tile([128, 512], f32)
                for i in range(2):
                    gg = 2 * qq + i
                    for hf in range(2):
                        nc.tensor.matmul(
                            ps[64 * hf : 64 * hf + 64, 256 * i : 256 * i + 256],
                            hh_sb[64 * hf : 64 * hf + 64, gg, :],
                            t2[64 * hf : 64 * hf + 64, gg, :],
                            start=True,
                            stop=True,
                        )
                y_sb = ysbp.tile([128, 2, 256], f16)
                src = ps[:].rearrange("p (i r) -> p i r", i=2)
                if qq % 2 == 0:
                    nc.vector.tensor_copy(y_sb[:], src)
                else:
                    nc.scalar.copy(y_sb[:], src)
                # all stores on sync: SP is idle during stage 2, and an
                # ACT store-issue would serialize behind its copies
                dst = y_d[qq].rearrange("h d k i r -> (h d k) i r")
                nc.sync.dma_start(dst, y_sb[:])

    nc.compile()
    return nc


def _pack_x1(x_rows):
    v = np.empty_like(x_rows)
    v[:, : N // 2] = x_rows[:, 0::2]
    v[:, N // 2 :] = x_rows[:, 1::2][:, ::-1]
    x1 = v.reshape(RPC, 128, 32).transpose(1, 2, 0).reshape(128, 8192)
    return np.ascontiguousarray(x1.astype(np.float16))


def kernel(x, _trace: bool = False):
    from concourse.bass_utils import run_bass_kernel_spmd

    x = np.asarray(x, dtype=np.float32)
    assert x.shape == (R, N)
    if "nc" not in _state:
        _state["nc"] = _build()
        _state["tables"] = _tables()
    nc = _state["nc"]
    f1_np, hh_np, k1_map = _state["tables"]

    in_maps = []
    for c in range(8):
        in_maps.append(
            {
                "x1": _pack_x1(x[c * RPC : (c + 1) * RPC]),
                "f1": f1_np,
                "hh": hh_np,
            }
        )

    res = run_bass_kernel_spmd(nc, in_maps, list(range(8)), trace=_trace)

    y = np.empty((R, N), dtype=np.float32)
    for c in range(8):
        ydev = res.results[c]["y"]  # [qq, hf, d, k2', i, r]
        perm = np.asarray(ydev, dtype=np.float32).transpose(5, 3, 1, 0, 4, 2)
        perm = perm.reshape(RPC, 32, 128)  # (r, k2', (a d)), a = 32hf+2qq+i
        yc = np.empty((RPC, 32, 128), dtype=np.float32)
        yc[:, :, k1_map] = perm
        y[c * RPC : (c + 1) * RPC] = yc.reshape(RPC, N)
    if _trace:
        _state["last_result"] = res
    return y


# revision 36
# speedup vs baseline: 1.0576x; 1.0576x over previous
"""FFT-based DCT-II on 8 trn2 NeuronCores (v5.1: radix 32x128, dual-half).

Per core (256 rows, no h-split): Makhoul DCT->real-FFT with n = 32*n1' +
n2'. Stage 1 contracts n1' with K=128 (full array), 128 slots s = 2a+m
(65 cos + 63 sin, cos_64 parked in the sin_0 slot). Mid transpose
roundtrips DRAM: write legs fully contiguous (t_dram == t_sb layout),
read legs 512B runs into a DUAL-HALF t2 (group-halves on partition
halves -> all 16 SDMA engines + stage-2 row/col quadrant packing, 4
concurrent matmuls). fp16 output, 1KB-contiguous stores.

Layouts:
  x1[n1', 256 n2' + r] = v[r, 32 n1' + n2']
  t_sb = t_dram [128 s=(2a+m), 32 n2', 256 r]
  t2   [128 = (hf, m, n2'), 32 gg, 256 r]   (group a = 32 hf + gg)
  hh   [128 = (hf, m, n2'), 32 gg, 64 = (d, k2')]
  y    [16 qq, 2 hf, 2 d, 32 k2', 2 i, 256 r], a = 32 hf + 2 qq + i
"""

import numpy as np

N = 4096
R = 2048
RPC = 256

_state = {}


def _tables():
    n1 = np.arange(128, dtype=np.float64)
    f1 = np.zeros((128, 128))
    a_ = np.arange(64, dtype=np.float64)
    f1[:, 0::2] = np.cos(2 * np.pi * n1[:, None] * a_[None, :] / 128)
    f1[:, 3::2] = -np.sin(2 * np.pi * n1[:, None] * a_[None, 1:] / 128)
    f1[:, 1] = np.cos(np.pi * n1)  # cos_64 in the sin_0 slot
    f1_np = f1.astype(np.float16)  # [128, 128]

    n2 = np.arange(32, dtype=np.float64)[:, None]
    k2 = np.arange(32, dtype=np.float64)[None, :]
    hh = np.zeros((128, 32, 64))
    for hf in range(2):
        for gg in range(32):
            a = 32 * hf + gg
            for d in range(2):
                k1 = (a if d == 0 else 128 - a) if a >= 1 else (
                    0 if d == 0 else 64
                )
                kk = 128 * k2 + k1
                th = np.pi * kk * (4 * n2 + 1) / 8192
                cols = (32 * d + np.arange(32))[None, :]
                rows = np.arange(32)[:, None]
                base = 64 * hf
                if a == 0:
                    hh[base + 32 * d + rows, gg, cols] = np.cos(th)
                else:
                    sgn = 1.0 if d == 0 else -1.0
                    hh[base + rows, gg, cols] = np.cos(th)
                    hh[base + 32 + rows, gg, cols] = sgn * np.sin(th)
    hh_np = hh.astype(np.float16).copy()  # [128, 32, 64]

    k1_map = np.empty(128, dtype=np.int64)
    for a in range(64):
        for d in range(2):
            k1_map[2 * a + d] = (a if d == 0 else 128 - a) if a >= 1 else (
                0 if d == 0 else 64
            )
    return f1_np, hh_np, k1_map


def _build():
    import concourse.tile as tile
    from concourse import bacc, mybir

    f16 = mybir.dt.float16
    f32 = mybir.dt.float32

    nc = bacc.Bacc("TRN2", target_bir_lowering=False, debug=False, num_devices=8)
    x1_d = nc.dram_tensor("x1", [128, 8192], f16, kind="ExternalInput").ap()
    f1_d = nc.dram_tensor("f1", [128, 128], f16, kind="ExternalInput").ap()
    hh_d = nc.dram_tensor("hh", [128, 32, 64], f16, kind="ExternalInput").ap()
    # [q2, hf, d, k2', qw, i, r]: qq = 2 q2 + qw; (qw i) adjacent so a
    # pair of qq-groups stores as one 3-dim AP with 2KB runs
    y_d = nc.dram_tensor(
        "y", [8, 2, 2, 32, 2, 2, 256], f16, kind="ExternalOutput"
    ).ap()

    with tile.TileContext(nc) as tc:
        with (
            tc.tile_pool(name="const", bufs=1) as const,
            tc.tile_pool(name="data", bufs=1) as data,
            tc.tile_pool(name="dram", bufs=1, space="DRAM") as dram,
            tc.tile_pool(name="ps1", bufs=4, space="PSUM") as ps1,
            tc.tile_pool(name="ps2", bufs=4, space="PSUM") as ps2,
            tc.tile_pool(name="ysb", bufs=12) as ysbp,
        ):
            f1_sb = const.tile([128, 128], f16)
            hh_sb = const.tile([128, 32, 64], f16)
            nc.sync.dma_start(f1_sb[:], f1_d)
            x1_g = []
            for g in range(8):
                xg = data.tile([128, 1024], f16, name=f"x1_{g}")
                nc.sync.dma_start(xg[:], x1_d[:, 1024 * g : 1024 * g + 1024])
                x1_g.append(xg)


            t_sb = data.tile([128, 32, 256], f16)
            t_dram = dram.tile([128, 32, 256], f16)  # same layout as t_sb
            t2 = data.tile([128, 32, 256], f16)  # (hf, m, n2'), gg, r

            # stage 1: per 2-n2' chunk one 1-bank psum tile, one matmul
            # [K=128, M=128, N=512], one copy out.
            cb = 0
            for g in range(8):
                for u in range(2):
                    ps = ps1.tile([128, 512], f32)
                    nc.tensor.matmul(
                        ps[:],
                        f1_sb[:],
                        x1_g[g][:, 512 * u : 512 * u + 512],
                        start=True,
                        stop=True,
                    )
                    n0 = 4 * g + 2 * u
                    dst = t_sb[:, n0 : n0 + 2, :]
                    src = ps[:].rearrange("p (n r) -> p n r", n=2)
                    if cb % 2 == 0:
                        nc.vector.tensor_copy(dst, src)
                    else:
                        nc.scalar.copy(dst, src)
                    cb += 1
                # transpose write legs: fully contiguous, eight 4-n2
                # waves, all on sync (SP is idle after the x issues;
                # keeps ACT free for copies)
                n0 = 4 * g
                nc.sync.dma_start(
                    t_dram[:, n0 : n0 + 4, :], t_sb[:, n0 : n0 + 4, :]
                )
                if g == 1:
                    # hh load on scalar, sequenced after two copies so
                    # its 0.5MB transfer misses both the x loads and the
                    # sync queue's write waves; lands well before stage 2
                    nc.scalar.dma_start(hh_sb[:], hh_d)

            # read legs: t2[64 hf + 32 m + n2', gg, r] =
            #            t_dram[2 (32 hf + gg) + m, n2', r]; 512B runs,
            # 32-partition dst, halves split across engine parity.
            t_dv = t_dram[:].rearrange("(a m) n r -> a m n r", m=2)
            for j4 in range(4):
                for hf in range(2):
                    for m in range(2):
                        src = t_dv[
                            32 * hf + 8 * j4 : 32 * hf + 8 * j4 + 8, m, :, :
                        ].rearrange("a n r -> n a r")
                        dst = t2[
                            64 * hf + 32 * m : 64 * hf + 32 * m + 32,
                            8 * j4 : 8 * j4 + 8,
                            :,
                        ]
                        if m == 0:
                            nc.sync.dma_start(dst, src)
                        else:
                            nc.scalar.dma_start(dst, src)

            # stage 2: per qq one 1-bank psum, 4 quadrant-packed matmuls
            # (group halves on row/col groups 0 and 64); copy; store.
            for qq in range(16):
                ps = ps2.tile([128, 512], f32)
                for i in range(2):
                    gg = 2 * qq + i
                    for hf in range(2):
                        nc.tensor.matmul(
                            ps[64 * hf : 64 * hf + 64, 256 * i : 256 * i + 256],
                            hh_sb[64 * hf : 64 * hf + 64, gg, :],
                            t2[64 * hf : 64 * hf + 64, gg, :],
                            start=True,
                            stop=True,
                        )
                if qq % 2 == 0:
                    y_sb = ysbp.tile([128, 2, 2, 256], f16)
                src = ps[:].rearrange("p (i r) -> p i r", i=2)
                if qq % 2 == 0:
                    nc.vector.tensor_copy(y_sb[:, 0, :, :], src)
                else:
                    nc.scalar.copy(y_sb[:, 1, :, :], src)
                    # paired store on sync: SP is idle during stage 2,
                    # and an ACT store-issue would serialize its copies
                    dst = y_d[qq // 2].rearrange(
                        "h d k q i r -> (h d k) (q i) r"
                    )
                    nc.sync.dma_start(
                        dst, y_sb[:].rearrange("p q i r -> p (q i) r")
                    )

    nc.compile()
    return nc


def _pack_x1(x_rows):
    v = np.empty_like(x_rows)
    v[:, : N // 2] = x_rows[:, 0::2]
    v[:, N // 2 :] = x_rows[:, 1::2][:, ::-1]
    x1 = v.reshape(RPC, 128, 32).transpose(1, 2, 0).reshape(128, 8192)
    return np.ascontiguousarray(x1.astype(np.float16))


def kernel(x, _trace: bool = False):
    from concourse.bass_utils import run_bass_kernel_spmd

    x = np.asarray(x, dtype=np.float32)
    assert x.shape == (R, N)
    if "nc" not in _state:
        _state["nc"] = _build()
        _state["tables"] = _tables()
    nc = _state["nc"]
    f1_np, hh_np, k1_map = _state["tables"]

    in_maps = []
    for c in range(8):
        in_maps.append(
            {
                "x1": _pack_x1(x[c * RPC : (c + 1) * RPC]),
                "f1": f1_np,
                "hh": hh_np,
            }
        )

    res = run_bass_kernel_spmd(nc, in_maps, list(range(8)), trace=_trace)

    y = np.empty((R, N), dtype=np.float32)
    for c in range(8):
        ydev = res.results[c]["y"]  # [q2, hf, d, k2', qw, i, r]
        # a = 32 hf + 2 (2 q2 + qw) + i -> axes (r, k2', hf, q2, qw, i, d)
        perm = np.asarray(ydev, dtype=np.float32).transpose(6, 3, 1, 0, 4, 5, 2)
        perm = perm.reshape(RPC, 32, 128)
        yc = np.empty((RPC, 32, 128), dtype=np.float32)
        yc[:, :, k1_map] = perm
        y[c * RPC : (c + 1) * RPC] = yc.reshape(RPC, N)
    if _trace:
        _state["last_result"] = res
    return y
